# revision 1
# baseline (speedup 1.0000x reference)
"""Trainium2 Bass kernel for nn_FeatureRotation.

Computes out[n, j, p, q] = sum_i W[i, j] * x[n, i, p, q] for
x: [64, 256, 56, 56] f32 and W: [256, 256] f32.

Sharding: data-parallel over the batch dim — 8 samples per core on 8
NeuronCores; W is replicated (or baked into the kernel structure).

Fast path: W produced by the reference's setup_inputs is an exact
permutation matrix (one-hot rows/cols), so the contraction is a channel
gather out[:, j] = x[:, src[j]] — pure data movement. Implemented as
DRAM->DRAM DMAs, with runs of consecutive channels (src[j+1] == src[j]+1)
coalesced into single strided DMAs. Multiplying by exact 0.0/1.0 and
summing zeros is exact in fp32, so this path is bit-exact with the einsum.

Fallback: if W is not exactly a permutation matrix, a dense TensorEngine
matmul kernel computes the contraction on-device.
"""

import os

import numpy as np

N, C, H, W_SP = 64, 256, 56, 56
HW = H * W_SP  # 3136
N_CORES = 8
NPC = N // N_CORES  # samples per core

_cache = {}
LAST_RESULTS = None  # BassKernelResults of the most recent device run


def _perm_source(Wm):
    """Return src with out[:, j] = x[:, src[j]] if Wm is exactly a
    permutation matrix, else None."""
    if Wm.shape != (C, C):
        return None
    if not np.all((Wm == 0.0) | (Wm == 1.0)):
        return None
    if not (np.all(Wm.sum(axis=0) == 1.0) and np.all(Wm.sum(axis=1) == 1.0)):
        return None
    return np.argmax(Wm, axis=0)


def _runs(src, max_len=256):
    """Maximal output-channel intervals whose sources are consecutive,
    optionally split to at most max_len channels per run."""
    runs = []
    j = 0
    while j < C:
        k = j
        while k + 1 < C and src[k + 1] == src[k] + 1 and (k + 1 - j) < max_len:
            k += 1
        runs.append((j, int(src[j]), k - j + 1))
        j = k + 1
    return runs


def _build_gather(runs):
    """Raw Bass kernel: one DRAM->DRAM DMA per run, all independent."""
    import concourse.bass as bass
    import concourse.mybir as mybir

    nc = bass.Bass("TRN2", target_bir_lowering=False)
    x = nc.dram_tensor("x", [NPC, C, HW], mybir.dt.float32, kind="ExternalInput")
    y = nc.dram_tensor("y", [NPC, C, HW], mybir.dt.float32, kind="ExternalOutput")
    sem = nc.alloc_semaphore()
    # Measured on HW: the HWDGE rings (sync/scalar) both map to SDMA
    # engines 64-71 only, while SWDGE (gpsimd) spreads every DMA across
    # all 16 engines (64-79) — so pure SWDGE maximizes pull bandwidth and
    # saturates the HBM stack (~630 GB/s read+write). hw_frac>0 would
    # move that share of bytes to the 8-engine HWDGE ring (never faster).
    hw_frac = float(os.environ.get("KERNEL_HW_FRAC", "0.0"))
    engines = [nc.gpsimd, nc.sync]
    ring_bytes = [0.0, 0.0]
    # Cap descriptors at one channel row (12544 B): measured marginally
    # faster than uncapped (94 vs 96 us) and strictly better than 6272.
    max_last = int(os.environ.get("KERNEL_MAX_LAST", "12544"))
    total = 0
    total_ch = sum(r[2] for r in runs)
    for dst, src0, L in sorted(runs, key=lambda r: -r[2]):
        ring = 1 if ring_bytes[1] + L <= hw_frac * total_ch else 0
        engines[ring].dma_start(
            y[:, dst : dst + L, :],
            x[:, src0 : src0 + L, :],
            # HWDGE sustains full rate on large descriptors; only SWDGE
            # benefits from the single-channel cap.
            max_dma_last_dim=None if ring == 1 else max_last,
        ).then_inc(sem, 16)
        ring_bytes[ring] += L
        total += 16
    nc.sync.wait_ge(sem, total)
    nc.gpsimd.wait_ge(sem, total)
    return nc


def _build_matmul():
    """Tile kernel: out[j, s] = sum_i W[i, j] x[i, s] per sample via PE."""
    import concourse.bacc as bacc
    import concourse.mybir as mybir
    from concourse.tile import TileContext

    f32 = mybir.dt.float32
    nc = bacc.Bacc("TRN2", target_bir_lowering=False)
    x = nc.dram_tensor("x", [NPC, C, HW], f32, kind="ExternalInput")
    w = nc.dram_tensor("w", [C, C], f32, kind="ExternalInput")
    y = nc.dram_tensor("y", [NPC, C, HW], f32, kind="ExternalOutput")
    SC = 448  # 3136 = 7 * 448; fits one PSUM bank in f32
    NS = HW // SC
    with TileContext(nc) as tc:
        with (
            tc.tile_pool(name="wpool", bufs=1) as wp,
            tc.tile_pool(name="xpool", bufs=6) as xp,
            tc.tile_pool(name="ppool", bufs=4, space="PSUM") as pp,
            tc.tile_pool(name="opool", bufs=4) as op,
        ):
            wt = []
            for ki in range(2):
                t = wp.tile([128, C], f32, tag=f"w{ki}")
                nc.sync.dma_start(t[:], w[ki * 128 : (ki + 1) * 128, :])
                wt.append(t)
            for n in range(NPC):
                for s in range(NS):
                    xts = []
                    for ki in range(2):
                        xt = xp.tile([128, SC], f32, tag="x")
                        nc.sync.dma_start(
                            xt[:],
                            x[n, ki * 128 : (ki + 1) * 128, s * SC : (s + 1) * SC],
                        )
                        xts.append(xt)
                    for m in range(2):
                        ps = pp.tile([128, SC], f32, tag="ps")
                        nc.tensor.matmul(
                            ps[:],
                            wt[0][:, m * 128 : (m + 1) * 128],
                            xts[0][:],
                            start=True,
                            stop=False,
                        )
                        nc.tensor.matmul(
                            ps[:],
                            wt[1][:, m * 128 : (m + 1) * 128],
                            xts[1][:],
                            start=False,
                            stop=True,
                        )
                        ot = op.tile([128, SC], f32, tag="o")
                        nc.vector.tensor_copy(ot[:], ps[:])
                        nc.sync.dma_start(
                            y[n, m * 128 : (m + 1) * 128, s * SC : (s + 1) * SC],
                            ot[:],
                        )
    nc.compile()  # Bacc defers register allocation to this pass
    return nc


def kernel(x, W):
    global LAST_RESULTS
    from concourse.bass_utils import run_bass_kernel_spmd

    x_np = np.ascontiguousarray(np.asarray(x), dtype=np.float32)
    W_np = np.ascontiguousarray(np.asarray(W), dtype=np.float32)
    xr = x_np.reshape(N, C, HW)

    src = _perm_source(W_np)
    if src is not None:
        key = ("gather", tuple(int(v) for v in src))
        if key not in _cache:
            max_len = int(os.environ.get("KERNEL_MAX_RUN", "256"))
            _cache[key] = _build_gather(_runs(src, max_len))
        nc = _cache[key]
        in_maps = [{"x": xr[c * NPC : (c + 1) * NPC]} for c in range(N_CORES)]
    else:
        if "matmul" not in _cache:
            _cache["matmul"] = _build_matmul()
        nc = _cache["matmul"]
        in_maps = [
            {"x": xr[c * NPC : (c + 1) * NPC], "w": W_np} for c in range(N_CORES)
        ]

    try:
        res = run_bass_kernel_spmd(nc, in_maps, core_ids=list(range(N_CORES)))
    except ModuleNotFoundError as e:
        if "axon_hooks" not in str(e):
            raise
        # BASS_TRACE was set but this image lacks the NTFF hook registry;
        # register an empty one (concourse then skips tracing) and retry.
        import sys
        import types

        import antenv

        mod = types.ModuleType("antenv.axon_hooks")
        mod.get_axon_ntff_profile_hook = lambda: None
        mod.set_axon_ntff_profile_hook = lambda h: None
        sys.modules["antenv.axon_hooks"] = mod
        antenv.axon_hooks = mod
        res = run_bass_kernel_spmd(nc, in_maps, core_ids=list(range(N_CORES)))
    LAST_RESULTS = res
    out = np.concatenate([r["y"] for r in res.results], axis=0)
    return out.reshape(N, C, H, W_SP)

